# revision 1
# baseline (speedup 1.0000x reference)
"""Causal self-attention (B=4, S=2048, D=1024, fp32, single head) on 8 TRN2 cores.

Sharding: core c -> (batch b=c//2, parity p=c%2). Each core owns 8 of the 16
q-tiles (128 rows each) of its batch, chosen so causal work is balanced:
parity 0 -> global q-tiles [0,3,4,7,8,11,12,15], parity 1 -> [1,2,5,6,9,10,13,14].
Both parities are run with padded kv-extents [2,4,6,8,10,12,14,16] (in sk-tiles)
per local q-tile so a single SPMD program serves all cores; causality/padding is
enforced by per-core mask *data* applied to the last two sk-tiles of each q-tile.

All matmuls in bf16 with fp32 PSUM accumulation. Host pre-transposes x/W
(so the kernel has no on-chip transposes), pre-folds the softmax scale into
q's projection, and folds bv/bp into one broadcast bias row:
out = softmax(qk^T/sqrt(D)) @ v @ Wp^T + bp
    = [ (E @ v_raw @ Wp^T) * (1/rowsum(E)) ] + (bp + Wp@bv),  E = exp(masked scores).
"""

import numpy as np
import ml_dtypes
from contextlib import ExitStack

import concourse.bass as bass
import concourse.mybir as mybir
from concourse import bacc
from concourse.tile import TileContext
from concourse.bass_utils import run_bass_kernel_spmd

P = 128
D = 1024
S = 2048
B = 4
NCORES = 8
SQL = S // 2            # local q rows per core
NE = D // P             # 8 e-subtiles of the embedding dim
NSK = S // P            # 16 sk tiles
GT = [[0, 3, 4, 7, 8, 11, 12, 15], [1, 2, 5, 6, 9, 10, 13, 14]]
PADK = [2, 4, 6, 8, 10, 12, 14, 16]   # padded kv sk-tiles per local q-tile
CHUNKS = [(0, 8), (1, 16)]            # (sq-chunk idx, padded sk-tiles)
SCALE = 1.0 / 32.0                    # 1/sqrt(D)
KV_GATHER = False                     # pair-wise AllGather kv-split

bf16 = mybir.dt.bfloat16
f32 = mybir.dt.float32
nbf = ml_dtypes.bfloat16
AF = mybir.ActivationFunctionType


def _chunk_start(c, j):
    """First valid local sq-subtile (0..3) of chunk c for sk-tile j."""
    return sum(1 for i in range(4) if PADK[c * 4 + i] <= j)


def build_nc(repeat=1, psa=6, psb=2, dma_split=True, xstb=3, epb=18, kv_gather=KV_GATHER, k_sinner=False, hint=False, apb=2, opb=3):
    nc = bacc.Bacc("TRN2", target_bir_lowering=False, num_devices=NCORES)

    if kv_gather:
        xT_h = nc.dram_tensor("xTh", [D, SQL], bf16, kind="ExternalInput")
    else:
        xT_h = nc.dram_tensor("xT", [D, S], bf16, kind="ExternalInput")
    xqT_h = nc.dram_tensor("xqT", [D, SQL], bf16, kind="ExternalInput")
    wk_h = nc.dram_tensor("wkT", [D, D], bf16, kind="ExternalInput")
    wv_h = nc.dram_tensor("wvT", [D, D], bf16, kind="ExternalInput")
    wq_h = nc.dram_tensor("wqT", [D, D], bf16, kind="ExternalInput")
    wp_h = nc.dram_tensor("wpT", [D, D], bf16, kind="ExternalInput")
    bk_h = nc.dram_tensor("bkT", [P, NE], f32, kind="ExternalInput")
    bq_h = nc.dram_tensor("bqT", [P, NE], f32, kind="ExternalInput")
    bp_h = nc.dram_tensor("bp_bc", [P, D], f32, kind="ExternalInput")
    mk_h = nc.dram_tensor("masks", [16, P, P], mybir.dt.uint8, kind="ExternalInput")
    out_h = nc.dram_tensor("out", [SQL, D], f32, kind="ExternalOutput")

    with TileContext(nc) as tc, ExitStack() as ctx:
        const = ctx.enter_context(tc.tile_pool(name="const", bufs=1))
        wpool = ctx.enter_context(tc.tile_pool(name="wpool", bufs=2))
        xst = ctx.enter_context(tc.tile_pool(name="xst", bufs=xstb))
        kpool = ctx.enter_context(tc.tile_pool(name="kpool", bufs=1))
        vpool = ctx.enter_context(tc.tile_pool(name="vpool", bufs=1))
        qpool = ctx.enter_context(tc.tile_pool(name="qpool", bufs=1))
        epool = ctx.enter_context(tc.tile_pool(name="epool", bufs=epb))
        apool = ctx.enter_context(tc.tile_pool(name="apool", bufs=apb))
        opool = ctx.enter_context(tc.tile_pool(name="opool", bufs=opb))
        rpool = ctx.enter_context(tc.tile_pool(name="rpool", bufs=8))
        psA = ctx.enter_context(tc.tile_pool(name="psA", bufs=psa, space="PSUM"))
        psB = ctx.enter_context(tc.tile_pool(name="psB", bufs=psb, space="PSUM"))
        drp = ctx.enter_context(tc.tile_pool(name="drp", bufs=1, space="DRAM"))
        stg = ctx.enter_context(tc.tile_pool(name="stg", bufs=2))

        rep_cm = tc.For_i(0, repeat, 1, hint_engines=tuple(nc.engines) if hint else ()) if repeat > 1 else None
        if rep_cm is not None:
            rep_cm.__enter__()
        for _rep in range(1):
            # constants
            bk_sb = const.tile([P, NE], f32, name="bk_sb", tag="bk")
            nc.gpsimd.dma_start(bk_sb, bk_h[:])
            bq_sb = const.tile([P, NE], f32, name="bq_sb", tag="bq")
            nc.gpsimd.dma_start(bq_sb, bq_h[:])
            bp_sb = const.tile([P, D], f32, name="bp_sb", tag="bp")
            nc.gpsimd.dma_start(bp_sb, bp_h[:])
            mk_sb = const.tile([P, 16, P], mybir.dt.uint8, name="mk_sb", tag="mk")
            nc.gpsimd.dma_start(mk_sb, mk_h[:].rearrange("i p q -> p i q"))
            ones_col = const.tile([P, 1], bf16, name="ones_col", tag="ones")
            nc.vector.memset(ones_col, 1.0)
            zeros_pp = const.tile([P, P], bf16, name="zeros_pp", tag="zpp")
            nc.vector.memset(zeros_pp, 0.0)

            # persistent per-core tensors (bf16)
            kT = kpool.tile([P, NE, S], bf16, name="kT_sb", tag="kT")
            v_sb = vpool.tile([P, NSK, D], bf16, name="v_sb", tag="v")
            qT = qpool.tile([P, NE, SQL], bf16, name="qT_sb", tag="qT")

            # ---- pass K ----
            wk = wpool.tile([P, NE, D], bf16, name="wk", tag="w")
            if kv_gather:
                # each core computes k,v in NATURAL [sk,e] layout for its own
                # half of the sequence, AllGathers with its pair core, then
                # reloads v directly and kT via bf16 DMA-transpose (+bk).
                for d_ in range(NE):
                    nc.sync.dma_start(wk[:, d_, :], wk_h[d_ * P:(d_ + 1) * P, :])
                wv = wpool.tile([P, NE, D], bf16, name="wvg", tag="w")
                for d_ in range(NE):
                    nc.sync.dma_start(wv[:, d_, :], wv_h[d_ * P:(d_ + 1) * P, :])
                halves = {}
                for which, wmat in (("k", wk), ("v", wv)):
                    hin = drp.tile([SQL, D], bf16, name=f"{which}_in", tag=f"{which}i")
                    for s in range(2):
                        xc = xst.tile([P, NE, 512], bf16, name=f"xc{which}{s}", tag="xt")
                        for d_ in range(NE):
                            nc.sync.dma_start(
                                xc[:, d_, :], xT_h[d_ * P:(d_ + 1) * P, s * 512:(s + 1) * 512]
                            )
                        for sv in range(4):
                            for n in range(2):
                                ps = psA.tile([P, 512], f32, name=f"pg{which}{s}_{sv}_{n}", tag="psA")
                                for d_ in range(NE):
                                    nc.tensor.matmul(
                                        ps, xc[:, d_, sv * P:(sv + 1) * P],
                                        wmat[:, d_, n * 512:(n + 1) * 512],
                                        start=(d_ == 0), stop=(d_ == NE - 1),
                                    )
                                hs = stg.tile([P, 512], bf16, name=f"hs{which}{s}_{sv}_{n}", tag="hs")
                                nc.vector.tensor_copy(hs, ps)
                                nc.sync.dma_start(
                                    hin[(s * 4 + sv) * P:(s * 4 + sv + 1) * P,
                                        n * 512:(n + 1) * 512], hs)
                    hout = drp.tile([S, D], bf16, name=f"{which}_gat", tag=f"{which}o")
                    nc.gpsimd.collective_compute(
                        "AllGather", mybir.AluOpType.bypass,
                        replica_groups=[[0, 1], [2, 3], [4, 5], [6, 7]],
                        ins=[hin[:]], outs=[hout[:]],
                    )
                    halves[which] = hout
                # reload v natural
                for j in range(NSK):
                    nc.sync.dma_start(v_sb[:, j, :], halves["v"][j * P:(j + 1) * P, :])
                # reload kT via DMA-transpose + bk bias
                for e in range(NE):
                    kstg = stg.tile([P, S], bf16, name=f"kstg{e}", tag="ks")
                    nc.sync.dma_start(
                        kstg, halves["k"][:, e * P:(e + 1) * P], transpose=True)
                    nc.scalar.activation(
                        kT[:, e, :], kstg, AF.Identity,
                        bias=bk_sb[:, e:e + 1], scale=1.0)
            # non-gather path: xc0 issued before wk; the (e=0,s=0) psum needs
            # xc0 in full but only one wk row-slice per d
            if k_sinner and not kv_gather:
                # s-chunk innermost: 4 consecutive matmuls share one stationary
                # wk slice (cuts per-matmul weight reloads); 2 psums per step
                xcs = []
                for s in range(4):
                    xc = xst.tile([P, NE, 512], bf16, name=f"xck{s}", tag="xt")
                    for d_ in range(NE):
                        nc.sync.dma_start(
                            xc[:, d_, :], xT_h[d_ * P:(d_ + 1) * P, s * 512:(s + 1) * 512]
                        )
                    if s == 0:
                        for d_ in range(NE):
                            nc.sync.dma_start(wk[:, d_, :], wk_h[d_ * P:(d_ + 1) * P, :])
                    xcs.append(xc)
                for e in range(NE):
                    for sh in range(2):       # s pairs to bound live psums
                        pss = [psA.tile([P, 512], f32, name=f"psk{e}_{sh}_{s2}", tag="psA")
                               for s2 in range(2)]
                        for d_ in range(NE):
                            for s2 in range(2):
                                nc.tensor.matmul(
                                    pss[s2], wk[:, d_, e * P:(e + 1) * P],
                                    xcs[sh * 2 + s2][:, d_, :],
                                    start=(d_ == 0), stop=(d_ == NE - 1),
                                )
                        for s2 in range(2):
                            nc.scalar.activation(
                                kT[:, e, (sh * 2 + s2) * 512:(sh * 2 + s2 + 1) * 512],
                                pss[s2], AF.Identity,
                                bias=bk_sb[:, e:e + 1], scale=1.0,
                            )
            for s in range(4 if not (kv_gather or k_sinner) else 0):
                xc = xst.tile([P, NE, 512], bf16, name=f"xck{s}", tag="xt")
                if dma_split:
                    for d_ in range(NE):
                        nc.sync.dma_start(
                            xc[:, d_, :], xT_h[d_ * P:(d_ + 1) * P, s * 512:(s + 1) * 512]
                        )
                    if s == 0:
                        for d_ in range(NE):
                            nc.sync.dma_start(wk[:, d_, :], wk_h[d_ * P:(d_ + 1) * P, :])
                else:
                    nc.sync.dma_start(
                        xc, xT_h[:, s * 512:(s + 1) * 512].rearrange("(ko p) f -> p ko f", p=P)
                    )
                    if s == 0:
                        nc.sync.dma_start(wk, wk_h[:].rearrange("(ko p) e -> p ko e", p=P))
                for e in range(NE):
                    ps = psA.tile([P, 512], f32, name=f"psk{s}_{e}", tag="psA")
                    for d_ in range(NE):
                        nc.tensor.matmul(
                            ps, wk[:, d_, e * P:(e + 1) * P], xc[:, d_, :],
                            start=(d_ == 0), stop=(d_ == NE - 1),
                        )
                    nc.scalar.activation(
                        kT[:, e, s * 512:(s + 1) * 512], ps, AF.Identity,
                        bias=bk_sb[:, e:e + 1], scale=1.0,
                    )

            # ---- pass V: v[sk, e] = xT chunks (stationary) x Wv^T ----
            wv = wpool.tile([P, NE, D], bf16, name="wv", tag="w") if not kv_gather else None
            if kv_gather:
                pass
            elif dma_split:
                for d_ in range(NE):
                    nc.sync.dma_start(wv[:, d_, :], wv_h[d_ * P:(d_ + 1) * P, :])
            else:
                nc.sync.dma_start(wv, wv_h[:].rearrange("(ko p) e -> p ko e", p=P))
            for s in range(4 if not kv_gather else 0):
                xc = xst.tile([P, NE, 512], bf16, name=f"xcv{s}", tag="xt")
                if dma_split:
                    for d_ in range(NE):
                        nc.sync.dma_start(
                            xc[:, d_, :], xT_h[d_ * P:(d_ + 1) * P, s * 512:(s + 1) * 512]
                        )
                else:
                    nc.sync.dma_start(
                        xc, xT_h[:, s * 512:(s + 1) * 512].rearrange("(ko p) f -> p ko f", p=P)
                    )
                for sv in range(4):
                    for n in range(2):
                        ps = psA.tile([P, 512], f32, name=f"psv{s}_{sv}_{n}", tag="psA")
                        for d_ in range(NE):
                            nc.tensor.matmul(
                                ps, xc[:, d_, sv * P:(sv + 1) * P],
                                wv[:, d_, n * 512:(n + 1) * 512],
                                start=(d_ == 0), stop=(d_ == NE - 1),
                            )
                        nc.vector.tensor_copy(
                            v_sb[:, s * 4 + sv, n * 512:(n + 1) * 512], ps
                        )

            # ---- pass Q: qT[e, sq] = Wq^T-tiles x xqT chunks (scale 1/32 folded) ----
            wq = wpool.tile([P, NE, D], bf16, name="wq", tag="w")
            if dma_split:
                for d_ in range(NE):
                    nc.sync.dma_start(wq[:, d_, :], wq_h[d_ * P:(d_ + 1) * P, :])
            else:
                nc.sync.dma_start(wq, wq_h[:].rearrange("(ko p) e -> p ko e", p=P))
            for c in range(2):
                xc = xst.tile([P, NE, 512], bf16, name=f"xcq{c}", tag="xt")
                if dma_split:
                    for d_ in range(NE):
                        nc.sync.dma_start(
                            xc[:, d_, :], xqT_h[d_ * P:(d_ + 1) * P, c * 512:(c + 1) * 512]
                        )
                else:
                    nc.sync.dma_start(
                        xc, xqT_h[:, c * 512:(c + 1) * 512].rearrange("(ko p) f -> p ko f", p=P)
                    )
                for e in range(NE):
                    ps = psA.tile([P, 512], f32, name=f"psq{c}_{e}", tag="psA")
                    for d_ in range(NE):
                        nc.tensor.matmul(
                            ps, wq[:, d_, e * P:(e + 1) * P], xc[:, d_, :],
                            start=(d_ == 0), stop=(d_ == NE - 1),
                        )
                    nc.scalar.activation(
                        qT[:, e, c * 512:(c + 1) * 512], ps, AF.Identity,
                        bias=bq_sb[:, e:e + 1], scale=SCALE,
                    )

            # ---- attention per sq-chunk ----
            wp = wpool.tile([P, NE, D], bf16, name="wp", tag="w")
            if dma_split:
                for d_ in range(NE):
                    nc.sync.dma_start(wp[:, d_, :], wp_h[d_ * P:(d_ + 1) * P, :])
            else:
                nc.sync.dma_start(wp, wp_h[:].rearrange("(ko p) e -> p ko e", p=P))

            for c, Kc in CHUNKS:
                # scoresT[j] = kT-tiles (stationary) x qT slice; then exp -> bf16
                exps = []
                for j in range(Kc):
                    s_off = _chunk_start(c, j) * P
                    Nj = 512 - s_off
                    ps = psA.tile([P, 512], f32, name=f"pss{c}_{j}", tag="psA")
                    psv = ps[:, :Nj]
                    for e in range(NE):
                        nc.tensor.matmul(
                            psv, kT[:, e, j * P:(j + 1) * P],
                            qT[:, e, c * 512 + s_off:(c + 1) * 512],
                            start=(e == 0), stop=(e == NE - 1),
                        )
                    ex = epool.tile([P, 512], bf16, name=f"exp{c}_{j}", tag="exp")
                    exv = ex[:, :Nj]
                    nc.scalar.activation(exv, psv, AF.Exp)
                    # mask the first sq-subtile where this j is diagonal/padding
                    mi = [i for i in range(4)
                          if PADK[c * 4 + i] - 2 == j or PADK[c * 4 + i] - 1 == j]
                    if mi:
                        i = mi[0]
                        which = int(PADK[c * 4 + i] - 1 == j)
                        gidx = 2 * (c * 4 + i) + which
                        nc.vector.copy_predicated(
                            ex[:, :P], mk_sb[:, gidx, :], zeros_pp
                        )
                    exps.append((exv, s_off, Nj))

                # denom^T[sq,1] per sq-subtile: exp-tiles (stationary) x ones
                recips = []
                for s4 in range(4):
                    Pi = PADK[c * 4 + s4]
                    pd = psB.tile([P, 1], f32, name=f"dn{c}_{s4}", tag="psB")
                    for j in range(Pi):
                        exv, s_off, _ = exps[j]
                        nc.tensor.matmul(
                            pd, exv[:, s4 * P - s_off:(s4 + 1) * P - s_off], ones_col,
                            start=(j == 0), stop=(j == Pi - 1),
                        )
                    rc = rpool.tile([P, 1], f32, name=f"rc{c}_{s4}", tag="rc")
                    nc.vector.reciprocal(rc, pd)
                    recips.append(rc)

                # AV: aoT[e, sq] accumulated over sk-tiles; v tiles stationary
                ao = apool.tile([P, NE, 512], bf16, name=f"ao{c}", tag="ao")
                for m in range(NE):
                    pa = psA.tile([P, 512], f32, name=f"pa{c}_{m}", tag="psA")
                    for j in range(Kc):
                        exv, s_off, _ = exps[j]
                        nc.tensor.matmul(
                            pa[:, s_off:512], v_sb[:, j, m * P:(m + 1) * P], exv,
                            start=(j == 0), stop=(j == Kc - 1),
                        )
                    nc.vector.tensor_copy(ao[:, m, :], pa)

                # proj: out[sq, e'] = aoT-tiles (stationary) x Wp^T; normalize + bias
                for s4 in range(4):
                    for n in range(2):
                        po = psA.tile([P, 512], f32, name=f"po{c}_{s4}_{n}", tag="psA")
                        for m in range(NE):
                            nc.tensor.matmul(
                                po, ao[:, m, s4 * P:(s4 + 1) * P],
                                wp[:, m, n * 512:(n + 1) * 512],
                                start=(m == 0), stop=(m == NE - 1),
                            )
                        ot = opool.tile([P, 512], f32, name=f"ot{c}_{s4}_{n}", tag="ot")
                        nc.vector.scalar_tensor_tensor(
                            ot, po, recips[s4], bp_sb[:, n * 512:(n + 1) * 512],
                            op0=mybir.AluOpType.mult, op1=mybir.AluOpType.add,
                        )
                        nc.sync.dma_start(
                            out_h[c * 512 + s4 * P:c * 512 + (s4 + 1) * P,
                                  n * 512:(n + 1) * 512],
                            ot,
                        )
        if rep_cm is not None:
            rep_cm.__exit__(None, None, None)
    nc.finalize()
    return nc


_NC_CACHE = None


def _get_nc():
    global _NC_CACHE
    if _NC_CACHE is None:
        _NC_CACHE = build_nc()
    return _NC_CACHE


def _prep_inputs(x, Wq, bq, Wk, bk, Wv, bv, Wp, bp):
    """Host-side shard prep: returns list of per-core input dicts."""
    x = np.asarray(x, np.float32)
    Wq, bq = np.asarray(Wq, np.float32), np.asarray(bq, np.float32)
    Wk, bk = np.asarray(Wk, np.float32), np.asarray(bk, np.float32)
    Wv, bv = np.asarray(Wv, np.float32), np.asarray(bv, np.float32)
    Wp, bp = np.asarray(Wp, np.float32), np.asarray(bp, np.float32)

    shared = {
        "wkT": np.ascontiguousarray(Wk.T).astype(nbf),
        "wvT": np.ascontiguousarray(Wv.T).astype(nbf),
        "wqT": np.ascontiguousarray(Wq.T).astype(nbf),
        "wpT": np.ascontiguousarray(Wp.T).astype(nbf),
        "bkT": np.ascontiguousarray(bk.reshape(NE, P).T),
        "bqT": np.ascontiguousarray((bq / 32.0).reshape(NE, P).T),
        "bp_bc": np.ascontiguousarray(
            np.tile((bp + Wp @ bv).astype(np.float32), (P, 1))
        ),
    }

    kk = np.arange(P)[:, None]
    qq = np.arange(P)[None, :]
    in_maps = []
    for c in range(NCORES):
        b, p = divmod(c, 2)
        g = GT[p]
        xb = x[b]
        if KV_GATHER:
            xT = np.ascontiguousarray(xb[p * SQL:(p + 1) * SQL].T).astype(nbf)
            xkey = "xTh"
        else:
            xT = np.ascontiguousarray(xb.T).astype(nbf)
            xkey = "xT"
        qrows = np.concatenate([xb[t * P:(t + 1) * P] for t in g], 0)
        xqT = np.ascontiguousarray(qrows.T).astype(nbf)
        masks = np.zeros((16, P, P), np.float32)
        for i in range(8):
            Pi, gi = PADK[i], g[i]
            for w, j in ((0, Pi - 2), (1, Pi - 1)):
                # 1.0 where DISALLOWED (key index > query index)
                masks[2 * i + w] = ((j * P + kk) > (gi * P + qq)).astype(np.float32)
        in_maps.append({
            xkey: xT, "xqT": xqT, "masks": masks.astype(np.uint8), **shared,
        })
    return in_maps


def _scatter_outputs(results):
    out = np.empty((B, S, D), np.float32)
    for c in range(NCORES):
        b, p = divmod(c, 2)
        o = results[c]["out"]
        for i, t in enumerate(GT[p]):
            out[b, t * P:(t + 1) * P] = o[i * P:(i + 1) * P]
    return out


def run(inputs, trace=False):
    nc = _get_nc()
    in_maps = _prep_inputs(**inputs)
    res = run_bass_kernel_spmd(
        nc, in_maps, core_ids=list(range(NCORES)), trace=trace
    )
    return _scatter_outputs(res.results), res


def kernel(**inputs):
    out, _ = run(inputs)
    return out



# revision 2
# speedup vs baseline: 18.8969x; 18.8969x over previous
"""Causal self-attention (B=4, S=2048, D=1024, fp32, single head) on 8 TRN2 cores — v2.

Sharding: core c -> (batch b=c//2, parity p=c%2). Attention q-tiles split by
parity as in v1 (balanced causal work). NEW vs v1:
  - K/V projections are computed for HALF the sequence per core (parity 0:
    sk-tiles 0-7, parity 1: 8-15) and exchanged pairwise via DRAM AllGather,
    removing the duplicated half of the K/V projection work (-25% PE).
  - All PSUM evacuations go through the Scalar (ACT) engine; DVE touches no
    PSUM tiles wider than 1 column (measured: DVE PSUM reads stall the PE).
  - Softmax denominators per chunk are one accumulated matmul chain with a
    stationary ones-column (out [1,512]), transposed to per-subtile columns
    with tiny fp32 matmuls; reciprocal scale is applied by ACT during the
    proj evacuation, and the bias row enters the proj PSUM as a rank-1
    (denom x bp_row) matmul, so no DVE pass over the output is needed.
"""

import numpy as np
import ml_dtypes
from contextlib import ExitStack

import concourse.bass as bass
import concourse.mybir as mybir
from concourse import bacc
from concourse.tile import TileContext
from concourse.bass_utils import run_bass_kernel_spmd

P = 128
D = 1024
S = 2048
B = 4
NCORES = 8
SQL = S // 2            # local q rows / kv-half rows per core
NE = D // P             # 8 e-subtiles of the embedding dim
NSK = S // P            # 16 sk tiles
GT = [[0, 3, 4, 7, 8, 11, 12, 15], [1, 2, 5, 6, 9, 10, 13, 14]]
PADK = [2, 4, 6, 8, 10, 12, 14, 16]   # padded kv sk-tiles per local q-tile
CHUNKS = [(0, 8), (1, 16)]            # (sq-chunk idx, padded sk-tiles)
SCALE = 1.0 / 32.0                    # 1/sqrt(D)

bf16 = mybir.dt.bfloat16
f32 = mybir.dt.float32
nbf = ml_dtypes.bfloat16
AF = mybir.ActivationFunctionType


def _chunk_start(c, j):
    """First valid local sq-subtile (0..3) of chunk c for sk-tile j."""
    return sum(1 for i in range(4) if PADK[c * 4 + i] <= j)


def build_nc(repeat=1, psa=7, psb=1, xstb=3, epb=18, apb=2, opb=3, stgb=4,
             mock_cc=False):
    nc = bacc.Bacc("TRN2", target_bir_lowering=False, num_devices=NCORES)

    xT_h = nc.dram_tensor("xTh", [D, SQL], bf16, kind="ExternalInput")
    xqT_h = nc.dram_tensor("xqT", [D, SQL], bf16, kind="ExternalInput")
    wk_h = nc.dram_tensor("wkT", [D, D], bf16, kind="ExternalInput")
    wv_h = nc.dram_tensor("wvT", [D, D], bf16, kind="ExternalInput")
    wq_h = nc.dram_tensor("wqT", [D, D], bf16, kind="ExternalInput")
    wp_h = nc.dram_tensor("wpT", [D, D], bf16, kind="ExternalInput")
    bk_h = nc.dram_tensor("bkT", [P, NE], f32, kind="ExternalInput")
    bq_h = nc.dram_tensor("bqT", [P, NE], f32, kind="ExternalInput")
    bp_h = nc.dram_tensor("bp_row", [1, D], bf16, kind="ExternalInput")
    mk_h = nc.dram_tensor("masks", [16, P, P], mybir.dt.uint8, kind="ExternalInput")
    out_h = nc.dram_tensor("out", [SQL, D], f32, kind="ExternalOutput")

    with TileContext(nc) as tc, ExitStack() as ctx:
        const = ctx.enter_context(tc.tile_pool(name="const", bufs=1))
        wpool = ctx.enter_context(tc.tile_pool(name="wpool", bufs=2))
        xst = ctx.enter_context(tc.tile_pool(name="xst", bufs=xstb))
        kpool = ctx.enter_context(tc.tile_pool(name="kpool", bufs=1))
        vpool = ctx.enter_context(tc.tile_pool(name="vpool", bufs=1))
        qpool = ctx.enter_context(tc.tile_pool(name="qpool", bufs=1))
        epool = ctx.enter_context(tc.tile_pool(name="epool", bufs=epb))
        apool = ctx.enter_context(tc.tile_pool(name="apool", bufs=apb))
        opool = ctx.enter_context(tc.tile_pool(name="opool", bufs=opb))
        rpool = ctx.enter_context(tc.tile_pool(name="rpool", bufs=10))
        rowp = ctx.enter_context(tc.tile_pool(name="rowp", bufs=2))
        stg = ctx.enter_context(tc.tile_pool(name="stg", bufs=stgb))
        psA = ctx.enter_context(tc.tile_pool(name="psA", bufs=psa, space="PSUM"))
        psB = ctx.enter_context(tc.tile_pool(name="psB", bufs=psb, space="PSUM"))
        drp = ctx.enter_context(tc.tile_pool(name="drp", bufs=1, space="DRAM"))
        drs = ctx.enter_context(tc.tile_pool(name="drs", bufs=1, space="DRAM"))

        rep_cm = tc.For_i(0, repeat, 1) if repeat > 1 else None
        if rep_cm is not None:
            rep_cm.__enter__()
        for _rep in range(1):
            # constants
            bk_sb = const.tile([P, NE], f32, name="bk_sb", tag="bk")
            nc.gpsimd.dma_start(bk_sb, bk_h[:])
            bq_sb = const.tile([P, NE], f32, name="bq_sb", tag="bq")
            nc.gpsimd.dma_start(bq_sb, bq_h[:])
            bp_sb = const.tile([1, D], bf16, name="bp_sb", tag="bp")
            nc.gpsimd.dma_start(bp_sb, bp_h[:])
            mk_sb = const.tile([P, 16, P], mybir.dt.uint8, name="mk_sb", tag="mk")
            nc.gpsimd.dma_start(mk_sb, mk_h[:].rearrange("i p q -> p i q"))
            ones_col = const.tile([P, 1], bf16, name="ones_col", tag="ones")
            nc.vector.memset(ones_col, 1.0)
            ones_f32 = const.tile([1, 1], f32, name="ones_f32", tag="onesf")
            nc.vector.memset(ones_f32, 1.0)
            zeros_pp = const.tile([P, P], bf16, name="zeros_pp", tag="zpp")
            nc.vector.memset(zeros_pp, 0.0)

            # persistent per-core tensors (bf16)
            kT = kpool.tile([P, NE, S], bf16, name="kT_sb", tag="kT")
            v_sb = vpool.tile([P, NSK, D], bf16, name="v_sb", tag="v")
            qT = qpool.tile([P, NE, SQL], bf16, name="qT_sb", tag="qT")

            # DRAM scratch for the pairwise K/V exchange
            khalf = drp.tile([NE, D, P], bf16, name="khalf", tag="kh")
            vhalf = drp.tile([SQL, D], bf16, name="vhalf", tag="vh")
            kgat = drs.tile([NSK, D, P], bf16, name="kgat", tag="kg")
            vgat = drs.tile([S, D], bf16, name="vgat", tag="vg")

            # ---- pass K (own half): kT-half[e, s] = Wk^T-tiles x xTh chunks ----
            wk = wpool.tile([P, NE, D], bf16, name="wk", tag="w")
            for s in range(2):
                xc = xst.tile([P, NE, 512], bf16, name=f"xck{s}", tag="xt")
                for d_ in range(NE):
                    nc.sync.dma_start(
                        xc[:, d_, :], xT_h[d_ * P:(d_ + 1) * P, s * 512:(s + 1) * 512]
                    )
                if s == 0:
                    for d_ in range(NE):
                        nc.sync.dma_start(wk[:, d_, :], wk_h[d_ * P:(d_ + 1) * P, :])
                for e in range(NE):
                    ps = psA.tile([P, 512], f32, name=f"psk{s}_{e}", tag="psA")
                    for d_ in range(NE):
                        nc.tensor.matmul(
                            ps, wk[:, d_, e * P:(e + 1) * P], xc[:, d_, :],
                            start=(d_ == 0), stop=(d_ == NE - 1),
                        )
                    ks = stg.tile([P, 512], bf16, name=f"ksg{s}_{e}", tag="stg")
                    nc.scalar.activation(
                        ks, ps, AF.Identity, bias=bk_sb[:, e:e + 1], scale=1.0)
                    nc.scalar.dma_start(
                        khalf[s * 4:(s + 1) * 4, e * P:(e + 1) * P, :]
                        .rearrange("t p i -> p t i"),
                        ks.rearrange("p (t i) -> p t i", t=4),
                    )
            if mock_cc:
                # timing-only stand-in: local DRAM copies with the same volume
                nc.gpsimd.dma_start(kgat[:2], khalf[:2])
                nc.gpsimd.dma_start(kgat[NE:NE + 2], khalf[:2])
            else:
                nc.gpsimd.collective_compute(
                    "AllGather", mybir.AluOpType.bypass,
                    replica_groups=[[0, 1], [2, 3], [4, 5], [6, 7]],
                    ins=[khalf[:]], outs=[kgat[:]],
                )

            # ---- pass V (own half): v[s, e] = xTh chunks (stationary) x Wv^T ----
            wv = wpool.tile([P, NE, D], bf16, name="wv", tag="w")
            for d_ in range(NE):
                nc.sync.dma_start(wv[:, d_, :], wv_h[d_ * P:(d_ + 1) * P, :])
            for s in range(2):
                xc = xst.tile([P, NE, 512], bf16, name=f"xcv{s}", tag="xt")
                for d_ in range(NE):
                    nc.sync.dma_start(
                        xc[:, d_, :], xT_h[d_ * P:(d_ + 1) * P, s * 512:(s + 1) * 512]
                    )
                for sv in range(4):
                    for n in range(2):
                        ps = psA.tile([P, 512], f32, name=f"psv{s}_{sv}_{n}", tag="psA")
                        for d_ in range(NE):
                            nc.tensor.matmul(
                                ps, xc[:, d_, sv * P:(sv + 1) * P],
                                wv[:, d_, n * 512:(n + 1) * 512],
                                start=(d_ == 0), stop=(d_ == NE - 1),
                            )
                        vs = stg.tile([P, 512], bf16, name=f"vsg{s}_{sv}_{n}", tag="stg")
                        nc.scalar.activation(vs, ps, AF.Identity)
                        nc.scalar.dma_start(
                            vhalf[(s * 4 + sv) * P:(s * 4 + sv + 1) * P,
                                  n * 512:(n + 1) * 512],
                            vs,
                        )
            if mock_cc:
                nc.gpsimd.dma_start(vgat[:SQL // 4], vhalf[:SQL // 4])
                nc.gpsimd.dma_start(vgat[SQL:SQL + SQL // 4], vhalf[:SQL // 4])
            else:
                nc.gpsimd.collective_compute(
                    "AllGather", mybir.AluOpType.bypass,
                    replica_groups=[[0, 1], [2, 3], [4, 5], [6, 7]],
                    ins=[vhalf[:]], outs=[vgat[:]],
                )

            # ---- pass Q: qT[e, sq] = Wq^T-tiles x xqT chunks (scale 1/32 folded) ----
            wq = wpool.tile([P, NE, D], bf16, name="wq", tag="w")
            for d_ in range(NE):
                nc.sync.dma_start(wq[:, d_, :], wq_h[d_ * P:(d_ + 1) * P, :])
            for c in range(2):
                xc = xst.tile([P, NE, 512], bf16, name=f"xcq{c}", tag="xt")
                for d_ in range(NE):
                    nc.sync.dma_start(
                        xc[:, d_, :], xqT_h[d_ * P:(d_ + 1) * P, c * 512:(c + 1) * 512]
                    )
                for e in range(NE):
                    ps = psA.tile([P, 512], f32, name=f"psq{c}_{e}", tag="psA")
                    for d_ in range(NE):
                        nc.tensor.matmul(
                            ps, wq[:, d_, e * P:(e + 1) * P], xc[:, d_, :],
                            start=(d_ == 0), stop=(d_ == NE - 1),
                        )
                    nc.scalar.activation(
                        qT[:, e, c * 512:(c + 1) * 512], ps, AF.Identity,
                        bias=bq_sb[:, e:e + 1], scale=SCALE,
                    )

            # readbacks of the gathered K/V (emitted late so their waits don't
            # head-of-line block earlier loads; split across SP and Pool queues)
            for e in range(NE):
                eng = nc.gpsimd
                eng.dma_start(
                    kT[:, e, :].rearrange("p (t i) -> p t i", t=NSK),
                    kgat[:, e * P:(e + 1) * P, :].rearrange("t p i -> p t i"),
                )
            for g in range(4):
                eng = nc.gpsimd
                eng.dma_start(
                    v_sb[:, g * 4:(g + 1) * 4, :],
                    vgat[g * 4 * P:(g + 1) * 4 * P, :]
                    .rearrange("(j p) e -> p j e", p=P),
                )

            # ---- attention per sq-chunk ----
            wp = wpool.tile([P, NE, D], bf16, name="wp", tag="w")
            for d_ in range(NE):
                nc.sync.dma_start(wp[:, d_, :], wp_h[d_ * P:(d_ + 1) * P, :])

            for c, Kc in CHUNKS:
                # scoresT[j] = kT-tiles (stationary) x qT slice; then exp -> bf16
                exps = []
                for j in range(Kc):
                    s_off = _chunk_start(c, j) * P
                    Nj = 512 - s_off
                    ps = psA.tile([P, 512], f32, name=f"pss{c}_{j}", tag="psA")
                    psv = ps[:, :Nj]
                    for e in range(NE):
                        nc.tensor.matmul(
                            psv, kT[:, e, j * P:(j + 1) * P],
                            qT[:, e, c * 512 + s_off:(c + 1) * 512],
                            start=(e == 0), stop=(e == NE - 1),
                        )
                    ex = epool.tile([P, 512], bf16, name=f"exp{c}_{j}", tag="exp")
                    exv = ex[:, :Nj]
                    nc.scalar.activation(exv, psv, AF.Exp)
                    # mask the first sq-subtile where this j is diagonal/padding
                    mi = [i for i in range(4)
                          if PADK[c * 4 + i] - 2 == j or PADK[c * 4 + i] - 1 == j]
                    if mi:
                        i = mi[0]
                        which = int(PADK[c * 4 + i] - 1 == j)
                        gidx = 2 * (c * 4 + i) + which
                        nc.vector.copy_predicated(
                            ex[:, :P], mk_sb[:, gidx, :], zeros_pp
                        )
                    exps.append((exv, s_off, Nj))

                # denomT row [1, 512]: ones-column stationary, exp tiles moving
                pd = psB.tile([1, 512], f32, name=f"pd{c}", tag="psB")
                for j in range(Kc):
                    exv, s_off, Nj = exps[j]
                    nc.tensor.matmul(
                        pd[:, s_off:512], ones_col, exv,
                        start=(j == 0), stop=(j == Kc - 1),
                    )
                pdrow_f = rowp.tile([1, 512], f32, name=f"pdf{c}", tag="pdf")
                nc.scalar.activation(pdrow_f, pd, AF.Identity)
                pdrow_b = rowp.tile([1, 512], bf16, name=f"pdb{c}", tag="pdb")
                nc.scalar.activation(pdrow_b, pd, AF.Identity)

                # transpose denom row to per-subtile columns; one DVE reciprocal
                rcp4 = psB.tile([P, 4], f32, name=f"rcp4_{c}", tag="psB")
                for s4 in range(4):
                    nc.tensor.matmul(
                        rcp4[:, s4:s4 + 1], pdrow_f[:, s4 * P:(s4 + 1) * P],
                        ones_f32, start=True, stop=True,
                    )
                rc4 = rpool.tile([P, 4], f32, name=f"rc4_{c}", tag="rc")
                nc.vector.reciprocal(rc4, rcp4)
                recips = [rc4[:, s4:s4 + 1] for s4 in range(4)]

                # AV: aoT[e, sq] accumulated over sk-tiles; v tiles stationary
                ao = apool.tile([P, NE, 512], bf16, name=f"ao{c}", tag="ao")
                for m in range(NE):
                    pa = psA.tile([P, 512], f32, name=f"pa{c}_{m}", tag="psA")
                    for j in range(Kc):
                        exv, s_off, _ = exps[j]
                        nc.tensor.matmul(
                            pa[:, s_off:512], v_sb[:, j, m * P:(m + 1) * P], exv,
                            start=(j == 0), stop=(j == Kc - 1),
                        )
                    nc.scalar.activation(ao[:, m, :], pa, AF.Identity)

                # proj: out[sq, e'] = aoT-tiles (stationary) x Wp^T
                # + rank-1 denom x bp_row into the same PSUM; ACT scales by recip
                for s4 in range(4):
                    for n in range(2):
                        po = psA.tile([P, 512], f32, name=f"po{c}_{s4}_{n}", tag="psA")
                        for m in range(NE):
                            nc.tensor.matmul(
                                po, ao[:, m, s4 * P:(s4 + 1) * P],
                                wp[:, m, n * 512:(n + 1) * 512],
                                start=(m == 0), stop=False,
                            )
                        nc.tensor.matmul(
                            po, pdrow_b[:, s4 * P:(s4 + 1) * P],
                            bp_sb[:, n * 512:(n + 1) * 512],
                            start=False, stop=True,
                        )
                        ot = opool.tile([P, 512], f32, name=f"ot{c}_{s4}_{n}", tag="ot")
                        nc.scalar.activation(
                            ot, po, AF.Identity, scale=recips[s4])
                        nc.gpsimd.dma_start(
                            out_h[c * 512 + s4 * P:c * 512 + (s4 + 1) * P,
                                  n * 512:(n + 1) * 512],
                            ot,
                        )
        if rep_cm is not None:
            rep_cm.__exit__(None, None, None)
    nc.finalize()
    return nc


_NC_CACHE = None


def _get_nc():
    global _NC_CACHE
    if _NC_CACHE is None:
        _NC_CACHE = build_nc()
    return _NC_CACHE


def _prep_inputs(x, Wq, bq, Wk, bk, Wv, bv, Wp, bp):
    """Host-side shard prep: returns list of per-core input dicts."""
    x = np.asarray(x, np.float32)
    Wq, bq = np.asarray(Wq, np.float32), np.asarray(bq, np.float32)
    Wk, bk = np.asarray(Wk, np.float32), np.asarray(bk, np.float32)
    Wv, bv = np.asarray(Wv, np.float32), np.asarray(bv, np.float32)
    Wp, bp = np.asarray(Wp, np.float32), np.asarray(bp, np.float32)

    shared = {
        "wkT": np.ascontiguousarray(Wk.T).astype(nbf),
        "wvT": np.ascontiguousarray(Wv.T).astype(nbf),
        "wqT": np.ascontiguousarray(Wq.T).astype(nbf),
        "wpT": np.ascontiguousarray(Wp.T).astype(nbf),
        "bkT": np.ascontiguousarray(bk.reshape(NE, P).T),
        "bqT": np.ascontiguousarray((bq / 32.0).reshape(NE, P).T),
        "bp_row": np.ascontiguousarray(
            (bp + Wp @ bv).astype(np.float32)[None, :]).astype(nbf),
    }

    kk = np.arange(P)[:, None]
    qq = np.arange(P)[None, :]
    in_maps = []
    for c in range(NCORES):
        b, p = divmod(c, 2)
        g = GT[p]
        xb = x[b]
        xT = np.ascontiguousarray(xb.T[:, p * SQL:(p + 1) * SQL]).astype(nbf)
        qrows = np.concatenate([xb[t * P:(t + 1) * P] for t in g], 0)
        xqT = np.ascontiguousarray(qrows.T).astype(nbf)
        masks = np.zeros((16, P, P), np.float32)
        for i in range(8):
            Pi, gi = PADK[i], g[i]
            for w, j in ((0, Pi - 2), (1, Pi - 1)):
                # 1.0 where DISALLOWED (key index > query index)
                masks[2 * i + w] = ((j * P + kk) > (gi * P + qq)).astype(np.float32)
        in_maps.append({
            "xTh": xT, "xqT": xqT, "masks": masks.astype(np.uint8), **shared,
        })
    return in_maps


def _scatter_outputs(results):
    out = np.empty((B, S, D), np.float32)
    for c in range(NCORES):
        b, p = divmod(c, 2)
        o = results[c]["out"]
        for i, t in enumerate(GT[p]):
            out[b, t * P:(t + 1) * P] = o[i * P:(i + 1) * P]
    return out


def run(inputs, trace=False):
    nc = _get_nc()
    in_maps = _prep_inputs(**inputs)
    res = run_bass_kernel_spmd(
        nc, in_maps, core_ids=list(range(NCORES)), trace=trace
    )
    return _scatter_outputs(res.results), res


def kernel(**inputs):
    out, _ = run(inputs)
    return out


# revision 4
# speedup vs baseline: 19.2518x; 1.0188x over previous
"""Causal self-attention (B=4, S=2048, D=1024, fp32, single head) on 8 TRN2 cores — v2.

Sharding: core c -> (batch b=c//2, parity p=c%2). Attention q-tiles split by
parity as in v1 (balanced causal work). NEW vs v1:
  - K/V projections are computed for HALF the sequence per core (parity 0:
    sk-tiles 0-7, parity 1: 8-15) and exchanged pairwise via DRAM AllGather,
    removing the duplicated half of the K/V projection work (-25% PE).
  - All PSUM evacuations go through the Scalar (ACT) engine; DVE touches no
    PSUM tiles wider than 1 column (measured: DVE PSUM reads stall the PE).
  - Softmax denominators per chunk are one accumulated matmul chain with a
    stationary ones-column (out [1,512]), transposed to per-subtile columns
    with tiny fp32 matmuls; reciprocal scale is applied by ACT during the
    proj evacuation, and the bias row enters the proj PSUM as a rank-1
    (denom x bp_row) matmul, so no DVE pass over the output is needed.
"""

import numpy as np
import ml_dtypes
from contextlib import ExitStack

import concourse.bass as bass
import concourse.mybir as mybir
from concourse import bacc
from concourse.tile import TileContext
from concourse.bass_utils import run_bass_kernel_spmd

P = 128
D = 1024
S = 2048
B = 4
NCORES = 8
SQL = S // 2            # local q rows / kv-half rows per core
NE = D // P             # 8 e-subtiles of the embedding dim
NSK = S // P            # 16 sk tiles
GT = [[0, 3, 4, 7, 8, 11, 12, 15], [1, 2, 5, 6, 9, 10, 13, 14]]
PADK = [2, 4, 6, 8, 10, 12, 14, 16]   # padded kv sk-tiles per local q-tile
CHUNKS = [(0, 8), (1, 16)]            # (sq-chunk idx, padded sk-tiles)
SCALE = 1.0 / 32.0                    # 1/sqrt(D)

bf16 = mybir.dt.bfloat16
f32 = mybir.dt.float32
nbf = ml_dtypes.bfloat16
AF = mybir.ActivationFunctionType


def _chunk_start(c, j):
    """First valid local sq-subtile (0..3) of chunk c for sk-tile j."""
    return sum(1 for i in range(4) if PADK[c * 4 + i] <= j)


def build_nc(repeat=1, psa=7, psb=1, xstb=3, epb=20, apb=3, opb=4, stgb=6,
             mock_cc=False):
    nc = bacc.Bacc("TRN2", target_bir_lowering=False, num_devices=NCORES)

    xT_h = nc.dram_tensor("xTh", [D, SQL], bf16, kind="ExternalInput")
    xqT_h = nc.dram_tensor("xqT", [D, SQL], bf16, kind="ExternalInput")
    wk_h = nc.dram_tensor("wkT", [D, D], bf16, kind="ExternalInput")
    wv_h = nc.dram_tensor("wvT", [D, D], bf16, kind="ExternalInput")
    wq_h = nc.dram_tensor("wqT", [D, D], bf16, kind="ExternalInput")
    wp_h = nc.dram_tensor("wpT", [D, D], bf16, kind="ExternalInput")
    bk_h = nc.dram_tensor("bkT", [P, NE], f32, kind="ExternalInput")
    bq_h = nc.dram_tensor("bqT", [P, NE], f32, kind="ExternalInput")
    bp_h = nc.dram_tensor("bp_row", [1, D], bf16, kind="ExternalInput")
    mk_h = nc.dram_tensor("masks", [16, P, P], mybir.dt.uint8, kind="ExternalInput")
    out_h = nc.dram_tensor("out", [SQL, D], f32, kind="ExternalOutput")

    with TileContext(nc) as tc, ExitStack() as ctx:
        const = ctx.enter_context(tc.tile_pool(name="const", bufs=1))
        wpool = ctx.enter_context(tc.tile_pool(name="wpool", bufs=2))
        xst = ctx.enter_context(tc.tile_pool(name="xst", bufs=xstb))
        kpool = ctx.enter_context(tc.tile_pool(name="kpool", bufs=1))
        vpool = ctx.enter_context(tc.tile_pool(name="vpool", bufs=1))
        qpool = ctx.enter_context(tc.tile_pool(name="qpool", bufs=1))
        epool = ctx.enter_context(tc.tile_pool(name="epool", bufs=epb))
        apool = ctx.enter_context(tc.tile_pool(name="apool", bufs=apb))
        opool = ctx.enter_context(tc.tile_pool(name="opool", bufs=opb))
        rpool = ctx.enter_context(tc.tile_pool(name="rpool", bufs=10))
        rowp = ctx.enter_context(tc.tile_pool(name="rowp", bufs=2))
        stg = ctx.enter_context(tc.tile_pool(name="stg", bufs=stgb))
        psA = ctx.enter_context(tc.tile_pool(name="psA", bufs=psa, space="PSUM"))
        psB = ctx.enter_context(tc.tile_pool(name="psB", bufs=psb, space="PSUM"))
        drp = ctx.enter_context(tc.tile_pool(name="drp", bufs=1, space="DRAM"))
        drs = ctx.enter_context(tc.tile_pool(name="drs", bufs=1, space="DRAM"))

        rep_cm = tc.For_i(0, repeat, 1) if repeat > 1 else None
        if rep_cm is not None:
            rep_cm.__enter__()
        for _rep in range(1):
            # constants
            bk_sb = const.tile([P, NE], f32, name="bk_sb", tag="bk")
            nc.gpsimd.dma_start(bk_sb, bk_h[:])
            bq_sb = const.tile([P, NE], f32, name="bq_sb", tag="bq")
            nc.gpsimd.dma_start(bq_sb, bq_h[:])
            bp_sb = const.tile([1, D], bf16, name="bp_sb", tag="bp")
            nc.gpsimd.dma_start(bp_sb, bp_h[:])
            mk_sb = const.tile([P, 16, P], mybir.dt.uint8, name="mk_sb", tag="mk")
            nc.gpsimd.dma_start(mk_sb, mk_h[:].rearrange("i p q -> p i q"))
            ones_col = const.tile([P, 1], bf16, name="ones_col", tag="ones")
            nc.vector.memset(ones_col, 1.0)
            ones_f32 = const.tile([1, 1], f32, name="ones_f32", tag="onesf")
            nc.vector.memset(ones_f32, 1.0)
            zeros_pp = const.tile([P, P], bf16, name="zeros_pp", tag="zpp")
            nc.vector.memset(zeros_pp, 0.0)

            # persistent per-core tensors (bf16)
            kT = kpool.tile([P, NE, S], bf16, name="kT_sb", tag="kT")
            v_sb = vpool.tile([P, NSK, D], bf16, name="v_sb", tag="v")
            qT = qpool.tile([P, NE, SQL], bf16, name="qT_sb", tag="qT")

            # DRAM scratch for the pairwise K/V exchange
            khalf = drp.tile([NE, D, P], bf16, name="khalf", tag="kh")
            vhalf = drp.tile([SQL, D], bf16, name="vhalf", tag="vh")
            kgat = drs.tile([NSK, D, P], bf16, name="kgat", tag="kg")
            vgat = drs.tile([S, D], bf16, name="vgat", tag="vg")

            # ---- pass K (own half): kT-half[e, s] = Wk^T-tiles x xTh chunks ----
            wk = wpool.tile([P, NE, D], bf16, name="wk", tag="w")
            for s in range(2):
                xc = xst.tile([P, NE, 512], bf16, name=f"xck{s}", tag="xt")
                for d_ in range(NE):
                    nc.sync.dma_start(
                        xc[:, d_, :], xT_h[d_ * P:(d_ + 1) * P, s * 512:(s + 1) * 512]
                    )
                if s == 0:
                    for d_ in range(NE):
                        nc.scalar.dma_start(wk[:, d_, :], wk_h[d_ * P:(d_ + 1) * P, :])
                for e in range(NE):
                    ps = psA.tile([P, 512], f32, name=f"psk{s}_{e}", tag="psA")
                    for d_ in range(NE):
                        nc.tensor.matmul(
                            ps, wk[:, d_, e * P:(e + 1) * P], xc[:, d_, :],
                            start=(d_ == 0), stop=(d_ == NE - 1),
                        )
                    ks = stg.tile([P, 512], bf16, name=f"ksg{s}_{e}", tag="stg")
                    nc.scalar.activation(
                        ks, ps, AF.Identity, bias=bk_sb[:, e:e + 1], scale=1.0)
                    nc.scalar.dma_start(
                        khalf[s * 4:(s + 1) * 4, e * P:(e + 1) * P, :]
                        .rearrange("t p i -> p t i"),
                        ks.rearrange("p (t i) -> p t i", t=4),
                    )
            if mock_cc:
                # timing-only stand-in: local DRAM copies with the same volume
                nc.gpsimd.dma_start(kgat[:2], khalf[:2])
                nc.gpsimd.dma_start(kgat[NE:NE + 2], khalf[:2])
            else:
                nc.gpsimd.collective_compute(
                    "AllGather", mybir.AluOpType.bypass,
                    replica_groups=[[0, 1], [2, 3], [4, 5], [6, 7]],
                    ins=[khalf[:]], outs=[kgat[:]],
                )

            # ---- pass V (own half): v[s, e] = xTh chunks (stationary) x Wv^T ----
            wv = wpool.tile([P, NE, D], bf16, name="wv", tag="w")
            for d_ in range(NE):
                nc.sync.dma_start(wv[:, d_, :], wv_h[d_ * P:(d_ + 1) * P, :])
            for s in range(2):
                xc = xst.tile([P, NE, 512], bf16, name=f"xcv{s}", tag="xt")
                for d_ in range(NE):
                    nc.sync.dma_start(
                        xc[:, d_, :], xT_h[d_ * P:(d_ + 1) * P, s * 512:(s + 1) * 512]
                    )
                for sv in range(4):
                    for n in range(2):
                        ps = psA.tile([P, 512], f32, name=f"psv{s}_{sv}_{n}", tag="psA")
                        for d_ in range(NE):
                            nc.tensor.matmul(
                                ps, xc[:, d_, sv * P:(sv + 1) * P],
                                wv[:, d_, n * 512:(n + 1) * 512],
                                start=(d_ == 0), stop=(d_ == NE - 1),
                            )
                        vs = stg.tile([P, 512], bf16, name=f"vsg{s}_{sv}_{n}", tag="stg")
                        nc.scalar.activation(vs, ps, AF.Identity)
                        nc.scalar.dma_start(
                            vhalf[(s * 4 + sv) * P:(s * 4 + sv + 1) * P,
                                  n * 512:(n + 1) * 512],
                            vs,
                        )
            if mock_cc:
                nc.gpsimd.dma_start(vgat[:SQL // 4], vhalf[:SQL // 4])
                nc.gpsimd.dma_start(vgat[SQL:SQL + SQL // 4], vhalf[:SQL // 4])
            else:
                nc.gpsimd.collective_compute(
                    "AllGather", mybir.AluOpType.bypass,
                    replica_groups=[[0, 1], [2, 3], [4, 5], [6, 7]],
                    ins=[vhalf[:]], outs=[vgat[:]],
                )

            # ---- pass Q: qT[e, sq] = Wq^T-tiles x xqT chunks (scale 1/32 folded) ----
            wq = wpool.tile([P, NE, D], bf16, name="wq", tag="w")
            for d_ in range(NE):
                nc.sync.dma_start(wq[:, d_, :], wq_h[d_ * P:(d_ + 1) * P, :])
            for c in range(2):
                xc = xst.tile([P, NE, 512], bf16, name=f"xcq{c}", tag="xt")
                for d_ in range(NE):
                    nc.sync.dma_start(
                        xc[:, d_, :], xqT_h[d_ * P:(d_ + 1) * P, c * 512:(c + 1) * 512]
                    )
                for e in range(NE):
                    ps = psA.tile([P, 512], f32, name=f"psq{c}_{e}", tag="psA")
                    for d_ in range(NE):
                        nc.tensor.matmul(
                            ps, wq[:, d_, e * P:(e + 1) * P], xc[:, d_, :],
                            start=(d_ == 0), stop=(d_ == NE - 1),
                        )
                    nc.scalar.activation(
                        qT[:, e, c * 512:(c + 1) * 512], ps, AF.Identity,
                        bias=bq_sb[:, e:e + 1], scale=SCALE,
                    )

            # readbacks of the gathered K/V (emitted late so their waits don't
            # head-of-line block earlier loads; split across SP and Pool queues)
            for t in range(NSK):
                nc.gpsimd.dma_start(
                    kT[:, :, t * P:(t + 1) * P],
                    kgat[t].rearrange("(e p) i -> p e i", p=P),
                )
            for g in range(4):
                eng = nc.gpsimd
                eng.dma_start(
                    v_sb[:, g * 4:(g + 1) * 4, :],
                    vgat[g * 4 * P:(g + 1) * 4 * P, :]
                    .rearrange("(j p) e -> p j e", p=P),
                )

            # ---- attention per sq-chunk ----
            wp = wpool.tile([P, NE, D], bf16, name="wp", tag="w")
            for d_ in range(NE):
                nc.sync.dma_start(wp[:, d_, :], wp_h[d_ * P:(d_ + 1) * P, :])

            for c, Kc in CHUNKS:
                # scoresT[j] = kT-tiles (stationary) x qT slice; then exp -> bf16
                exps = []
                for j in range(Kc):
                    s_off = _chunk_start(c, j) * P
                    Nj = 512 - s_off
                    ps = psA.tile([P, 512], f32, name=f"pss{c}_{j}", tag="psA")
                    psv = ps[:, :Nj]
                    for e in range(NE):
                        nc.tensor.matmul(
                            psv, kT[:, e, j * P:(j + 1) * P],
                            qT[:, e, c * 512 + s_off:(c + 1) * 512],
                            start=(e == 0), stop=(e == NE - 1),
                        )
                    ex = epool.tile([P, 512], bf16, name=f"exp{c}_{j}", tag="exp")
                    exv = ex[:, :Nj]
                    nc.scalar.activation(exv, psv, AF.Exp)
                    # mask the first sq-subtile where this j is diagonal/padding
                    mi = [i for i in range(4)
                          if PADK[c * 4 + i] - 2 == j or PADK[c * 4 + i] - 1 == j]
                    if mi:
                        i = mi[0]
                        which = int(PADK[c * 4 + i] - 1 == j)
                        gidx = 2 * (c * 4 + i) + which
                        nc.vector.copy_predicated(
                            ex[:, :P], mk_sb[:, gidx, :], zeros_pp
                        )
                    exps.append((exv, s_off, Nj))

                # denomT row [1, 512]: ones-column stationary, exp tiles moving
                pd = psB.tile([1, 512], f32, name=f"pd{c}", tag="psB")
                for j in range(Kc):
                    exv, s_off, Nj = exps[j]
                    nc.tensor.matmul(
                        pd[:, s_off:512], ones_col, exv,
                        start=(j == 0), stop=(j == Kc - 1),
                    )
                pdrow_f = rowp.tile([1, 512], f32, name=f"pdf{c}", tag="pdf")
                nc.scalar.activation(pdrow_f, pd, AF.Identity)
                pdrow_b = rowp.tile([1, 512], bf16, name=f"pdb{c}", tag="pdb")
                nc.scalar.activation(pdrow_b, pd, AF.Identity)

                # transpose denom row to per-subtile columns; one DVE reciprocal
                rcp4 = psB.tile([P, 4], f32, name=f"rcp4_{c}", tag="psB")
                for s4 in range(4):
                    nc.tensor.matmul(
                        rcp4[:, s4:s4 + 1], pdrow_f[:, s4 * P:(s4 + 1) * P],
                        ones_f32, start=True, stop=True,
                    )
                rc4 = rpool.tile([P, 4], f32, name=f"rc4_{c}", tag="rc")
                nc.vector.reciprocal(rc4, rcp4)
                recips = [rc4[:, s4:s4 + 1] for s4 in range(4)]

                # AV: aoT[e, sq] accumulated over sk-tiles; v tiles stationary
                ao = apool.tile([P, NE, 512], bf16, name=f"ao{c}", tag="ao")
                for m in range(NE):
                    pa = psA.tile([P, 512], f32, name=f"pa{c}_{m}", tag="psA")
                    for j in range(Kc):
                        exv, s_off, _ = exps[j]
                        nc.tensor.matmul(
                            pa[:, s_off:512], v_sb[:, j, m * P:(m + 1) * P], exv,
                            start=(j == 0), stop=(j == Kc - 1),
                        )
                    nc.scalar.activation(ao[:, m, :], pa, AF.Identity)

                # proj: out[sq, e'] = aoT-tiles (stationary) x Wp^T
                # + rank-1 denom x bp_row into the same PSUM; ACT scales by recip
                for s4 in range(4):
                    for n in range(2):
                        po = psA.tile([P, 512], f32, name=f"po{c}_{s4}_{n}", tag="psA")
                        for m in range(NE):
                            nc.tensor.matmul(
                                po, ao[:, m, s4 * P:(s4 + 1) * P],
                                wp[:, m, n * 512:(n + 1) * 512],
                                start=(m == 0), stop=False,
                            )
                        nc.tensor.matmul(
                            po, pdrow_b[:, s4 * P:(s4 + 1) * P],
                            bp_sb[:, n * 512:(n + 1) * 512],
                            start=False, stop=True,
                        )
                        ot = opool.tile([P, 512], f32, name=f"ot{c}_{s4}_{n}", tag="ot")
                        nc.scalar.activation(
                            ot, po, AF.Identity, scale=recips[s4])
                        nc.gpsimd.dma_start(
                            out_h[c * 512 + s4 * P:c * 512 + (s4 + 1) * P,
                                  n * 512:(n + 1) * 512],
                            ot,
                        )
        if rep_cm is not None:
            rep_cm.__exit__(None, None, None)
    nc.finalize()
    return nc


_NC_CACHE = None


def _get_nc():
    global _NC_CACHE
    if _NC_CACHE is None:
        _NC_CACHE = build_nc()
    return _NC_CACHE


def _prep_inputs(x, Wq, bq, Wk, bk, Wv, bv, Wp, bp):
    """Host-side shard prep: returns list of per-core input dicts."""
    x = np.asarray(x, np.float32)
    Wq, bq = np.asarray(Wq, np.float32), np.asarray(bq, np.float32)
    Wk, bk = np.asarray(Wk, np.float32), np.asarray(bk, np.float32)
    Wv, bv = np.asarray(Wv, np.float32), np.asarray(bv, np.float32)
    Wp, bp = np.asarray(Wp, np.float32), np.asarray(bp, np.float32)

    shared = {
        "wkT": np.ascontiguousarray(Wk.T).astype(nbf),
        "wvT": np.ascontiguousarray(Wv.T).astype(nbf),
        "wqT": np.ascontiguousarray(Wq.T).astype(nbf),
        "wpT": np.ascontiguousarray(Wp.T).astype(nbf),
        "bkT": np.ascontiguousarray(bk.reshape(NE, P).T),
        "bqT": np.ascontiguousarray((bq / 32.0).reshape(NE, P).T),
        "bp_row": np.ascontiguousarray(
            (bp + Wp @ bv).astype(np.float32)[None, :]).astype(nbf),
    }

    kk = np.arange(P)[:, None]
    qq = np.arange(P)[None, :]
    in_maps = []
    for c in range(NCORES):
        b, p = divmod(c, 2)
        g = GT[p]
        xb = x[b]
        xT = np.ascontiguousarray(xb.T[:, p * SQL:(p + 1) * SQL]).astype(nbf)
        qrows = np.concatenate([xb[t * P:(t + 1) * P] for t in g], 0)
        xqT = np.ascontiguousarray(qrows.T).astype(nbf)
        masks = np.zeros((16, P, P), np.float32)
        for i in range(8):
            Pi, gi = PADK[i], g[i]
            for w, j in ((0, Pi - 2), (1, Pi - 1)):
                # 1.0 where DISALLOWED (key index > query index)
                masks[2 * i + w] = ((j * P + kk) > (gi * P + qq)).astype(np.float32)
        in_maps.append({
            "xTh": xT, "xqT": xqT, "masks": masks.astype(np.uint8), **shared,
        })
    return in_maps


def _scatter_outputs(results):
    out = np.empty((B, S, D), np.float32)
    for c in range(NCORES):
        b, p = divmod(c, 2)
        o = results[c]["out"]
        for i, t in enumerate(GT[p]):
            out[b, t * P:(t + 1) * P] = o[i * P:(i + 1) * P]
    return out


def run(inputs, trace=False):
    nc = _get_nc()
    in_maps = _prep_inputs(**inputs)
    res = run_bass_kernel_spmd(
        nc, in_maps, core_ids=list(range(NCORES)), trace=trace
    )
    return _scatter_outputs(res.results), res


def kernel(**inputs):
    out, _ = run(inputs)
    return out
